# revision 30
# baseline (speedup 1.0000x reference)
"""Trainium2 Bass kernel for FBPINN-with-window (dense MoE over 16 subnets).

Math (per point n):
    h   = relu(x @ pW0 + pb0); h += relu(h @ pWmid_l + pbmid_l) (x2)
    z   = h @ pWl + pbl;  ez = exp(z)            (softmax un-normalized)
    xn_c = (x - center_c)/scale_c  (folded on host into layer-0 weights)
    g_c = tanh(xn_c @ W0_c + b0_c); g_c = tanh(g_c @ Wmid_cl + bmid_cl) (x2)
    u_c = g_c @ Wl_c + bl_c
    acc = sum_c softmax(z)_c * u_c = (sum_c ez_c*(g_c@Wl_c) + sum_c ez_c*bl_c) / sum_c ez_c
    out = acc * x0(1-x0)*x1(1-x1)

Device layout: activations transposed (features on partitions, points on the
free dim).  Data-parallel over 8 cores (8192 points each), 4 point-tiles of
2048 per core.  Subnet activations and mid/last weights are fp16 (rel err
~5e-4 vs fp32 — verified against the reference on host); all weights are
SBUF-resident, no streaming.

The Scalar (tanh) and Tensor engines are both ~790us of intrinsic work per
core, so the schedule keeps them co-saturated: PSUM is run as a ring of four
[128, 1024] half-tiles (one PE half-fill ~1.06us matches one half-tanh
~1.02us), and the per-subnet 7-step chains (l0 fc0/fc1, 4 mids, last) are
software-pipelined with a 3-slot skew so every window mixes ACT-heavy l0,
balanced mid, and PE-only last steps from different subnets.  Last layers
write [1, PTILE] PSUM rows that a DVE multiply folds the softmax weight into.
"""

import os

import numpy as np

N = 65536
D = 2
C = 16
PH = 128
PNMID = 2
SW = 256
SNMID = 2

NCORES = 8
NP = N // NCORES          # 8192 points per core
PTILE = 2048              # points per tile
NT = NP // PTILE          # 4 tiles
CHK = 512                 # matmul moving free dim (one PSUM bank)
NCH = PTILE // CHK        # 4 chunks per tile
FC = SW // 128            # 2 feature chunks
KC = SW // 128            # 2 contraction chunks
PPB = NP // 128           # 64 points per partition (points-layout)
HP = PTILE // 2           # psum half-tile (2 banks): 4-slot psum ring
NHH = HP // CHK           # 2 chunks per half
WBLK = SNMID * FC * KC * 128  # mid-weight cols per subnet (1024)

_CACHE = {}


def _build():
    import concourse.mybir as mybir
    import concourse.tile as tile
    from concourse import bacc

    f32 = mybir.dt.float32
    f32r = mybir.dt.float32r
    f16 = mybir.dt.float16
    AF = mybir.ActivationFunctionType
    OP = mybir.AluOpType

    nc = bacc.Bacc("TRN2", debug=False)

    def din(name, shape, dt=f32):
        return nc.dram_tensor(name, shape, dt, kind="ExternalInput").ap()

    xT = din("xT", (3, NP), f32r)
    xP = din("xP", (128, 2 * PPB))
    w0q = din("w0q", (128, C * FC * 128), f32r)
    wm = din("wm", (128, C * WBLK), f16)
    bm = din("bm", (128, C * SNMID * FC))
    wl = din("wl", (128, C * KC), f16)
    pw0q = din("pw0q", (128, PH), f32r)
    pwm = din("pwm", (PH, PNMID * PH), f32r)
    pbm = din("pbm", (PH, PNMID))
    pwl = din("pwl", (PH, C), f32r)
    pbl = din("pbl", (C, 1))
    cw = din("cw", (C, 2), f16)
    onec = din("onec", (C, 1), f32r)
    y = nc.dram_tensor("y", (NP,), f32, kind="ExternalOutput").ap()

    with tile.TileContext(nc) as tc:
        with (
            tc.tile_pool(name="wp", bufs=1) as wp,
            tc.tile_pool(name="gp", bufs=6) as gp,
            tc.tile_pool(name="hp", bufs=2) as hp,
            tc.tile_pool(name="sp", bufs=2) as sp,
            tc.tile_pool(name="rp", bufs=2) as rp,
            tc.tile_pool(name="xp", bufs=2) as xpl,
            tc.tile_pool(name="fin", bufs=1) as fin,
            tc.tile_pool(name="pp", bufs=4, space="PSUM") as pp,
        ):
            # ---- weights / constants into SBUF (all resident) ----
            s_pw0q = wp.tile([128, PH], f32r)
            nc.sync.dma_start(s_pw0q[:], pw0q)
            s_pwm = wp.tile([PH, PNMID * PH], f32r)
            nc.sync.dma_start(s_pwm[:], pwm)
            s_pbm = wp.tile([PH, PNMID], f32)
            nc.sync.dma_start(s_pbm[:], pbm)
            s_pwl = wp.tile([PH, C], f32r)
            nc.sync.dma_start(s_pwl[:], pwl)
            s_pbl = wp.tile([C, 1], f32)
            nc.sync.dma_start(s_pbl[:], pbl)
            s_cw = wp.tile([C, 2], f16)
            nc.sync.dma_start(s_cw[:], cw)
            s_one = wp.tile([C, 1], f32r)
            nc.sync.dma_start(s_one[:], onec)
            s_w0q = wp.tile([128, C * FC * 128], f32r)
            for cq in range(4):
                qb = C * FC * 32
                nc.sync.dma_start(
                    s_w0q[:, cq * qb : (cq + 1) * qb], w0q[:, cq * qb : (cq + 1) * qb]
                )
            # tile-0 x layout before the big mid-weight load: it is on the
            # critical path of the first l0 matmuls
            xt4_0 = wp.tile([128, PTILE], f32r)
            for rr in range(NCH):
                nc.sync.dma_start(
                    xt4_0[32 * rr : 32 * rr + 3, rr * CHK : (rr + 1) * CHK],
                    xT[:, rr * CHK : (rr + 1) * CHK],
                )
            s_bm = wp.tile([128, C * SNMID * FC], f32)
            nc.sync.dma_start(s_bm[:], bm)
            s_wl = wp.tile([128, C * KC], f16)
            nc.sync.dma_start(s_wl[:], wl)
            s_wm = wp.tile([128, C * WBLK], f16)
            for cq in range(16):
                qb = C * WBLK // 16
                nc.sync.dma_start(
                    s_wm[:, cq * qb : (cq + 1) * qb], wm[:, cq * qb : (cq + 1) * qb]
                )

            # ---- per-core x (points-layout) + boundary factor ----
            s_xP = fin.tile([128, 2 * PPB], f32)
            nc.sync.dma_start(s_xP[:], xP)
            s_xmx = fin.tile([128, 2 * PPB], f32)
            nc.vector.tensor_mul(s_xmx[:], s_xP[:], s_xP[:])
            nc.vector.tensor_sub(s_xmx[:], s_xP[:], s_xmx[:])
            v = s_xmx.rearrange("p (j two) -> p j two", two=2)
            s_bc = fin.tile([128, PPB], f32)
            nc.vector.tensor_mul(s_bc[:], v[:, :, 0], v[:, :, 1])

            # points-layout accumulators, filled per tile via reshape DMAs
            s_accP = fin.tile([128, PPB], f32)
            s_s1P = fin.tile([128, PPB], f32)
            s_s2P = fin.tile([128, PPB], f32)

            # ---------- skewed software pipeline over (tile, subnet) chains ----
            # Chain j = t*C + c runs its 7 steps (l0 fc0/fc1, 4 mids, last)
            # at slots OFF*j + k.  Each slot then mixes l0 (ACT-heavy), mid
            # (balanced) and last (PE-only) steps of different subnets, so
            # both engines see near-constant load instead of phase bursts.
            OFF = 3
            NJ = NT * C
            tiles = {}
            xfetched = {}

            def pou_l0(ts):
                h = hp.tile([PH, PTILE], f32r, tag="h", name="h")
                for hf in range(2):
                    ps0 = pp.tile([PH, HP], f32, tag="mm", name="ps0")
                    for q in range(NHH):
                        rr = hf * NHH + q
                        nc.tensor.matmul(
                            ps0[:, q * CHK : (q + 1) * CHK],
                            s_pw0q[32 * rr : 32 * rr + 3, :],
                            ts["xt4"][32 * rr : 32 * rr + 3, rr * CHK : (rr + 1) * CHK],
                            start=True,
                            stop=True,
                            tile_position=(32 * rr, 0),
                        )
                    nc.vector.tensor_scalar_max(h[:, hf * HP : (hf + 1) * HP], ps0[:], 0.0)
                ts["h"] = h

            def pou_mid(ts, l):
                h = ts["h"]
                hr = hp.tile([PH, PTILE], f32r, tag="h", name="hr")
                for hf in range(2):
                    psl = pp.tile([PH, HP], f32, tag="mm", name="psl")
                    for q in range(NHH):
                        n = hf * NHH + q
                        nc.tensor.matmul(
                            psl[:, q * CHK : (q + 1) * CHK],
                            s_pwm[:, l * PH : (l + 1) * PH],
                            h[:, n * CHK : (n + 1) * CHK],
                            start=True,
                            stop=True,
                        )
                    nc.vector.tensor_scalar(
                        hr[:, hf * HP : (hf + 1) * HP], psl[:],
                        s_pbm[:, l : l + 1], 0.0, op0=OP.add, op1=OP.max
                    )
                nc.vector.tensor_add(hr[:], hr[:], h[:])
                ts["h"] = hr

            def pou_logits(ts):
                h = ts["h"]
                ez = sp.tile([C, PTILE], f16, tag="ez", name="ez")
                for hf in range(2):
                    psz = pp.tile([C, HP], f32, tag="mm", name="psz")
                    for q in range(NHH):
                        n = hf * NHH + q
                        nc.tensor.matmul(
                            psz[:, q * CHK : (q + 1) * CHK],
                            s_pwl[:],
                            h[:, n * CHK : (n + 1) * CHK],
                            start=True,
                            stop=True,
                        )
                    nc.scalar.activation(
                        ez[:, hf * HP : (hf + 1) * HP], psz[:], AF.Exp,
                        bias=s_pbl[:, 0:1],
                    )
                ts["ez"] = ez

            def pou_s12(ts):
                t = ts["t"]
                s12row = rp.tile([2, PTILE], f32, tag="row", name="s12row")
                for hf in range(2):
                    pss = pp.tile([2, HP], f32, tag="mm", name="pss")
                    for q in range(NHH):
                        n = hf * NHH + q
                        nc.tensor.matmul(
                            pss[:, q * CHK : (q + 1) * CHK],
                            s_cw[:],
                            ts["ez"][:, n * CHK : (n + 1) * CHK],
                            start=True,
                            stop=True,
                        )
                    nc.vector.tensor_copy(s12row[:, hf * HP : (hf + 1) * HP], pss[:])
                nc.sync.dma_start(s_s1P[t * 32 : (t + 1) * 32, :], s12row[0:1, :])
                nc.sync.dma_start(s_s2P[t * 32 : (t + 1) * 32, :], s12row[1:2, :])

            def sub_l0(ts, c, st, fc):
                if fc == 0:
                    st["g0"] = gp.tile([128, KC, PTILE], f16, tag="g", name="g0")
                col = (c * FC + fc) * 128
                for hf in range(2):
                    pt = pp.tile([128, HP], f32, tag="mm", name="pt")
                    for q in range(NHH):
                        rr = hf * NHH + q
                        nc.tensor.matmul(
                            pt[:, q * CHK : (q + 1) * CHK],
                            s_w0q[32 * rr : 32 * rr + 3, col : col + 128],
                            ts["xt4"][32 * rr : 32 * rr + 3, rr * CHK : (rr + 1) * CHK],
                            start=True,
                            stop=True,
                            tile_position=(32 * rr, 0),
                        )
                    nc.scalar.activation(
                        st["g0"][:, fc, hf * HP : (hf + 1) * HP], pt[:], AF.Tanh
                    )

            def sub_mid(c, st, l, fc):
                gcur = st[f"g{l}"]
                if fc == 0:
                    st[f"g{l + 1}"] = gp.tile(
                        [128, KC, PTILE], f16, tag="g", name=f"g{l + 1}"
                    )
                bcol = (c * SNMID + l) * FC + fc
                for hf in range(2):
                    pt = pp.tile([128, HP], f32, tag="mm", name="pt")
                    for kc in range(KC):
                        col = c * WBLK + ((l * FC + fc) * KC + kc) * 128
                        for q in range(NHH):
                            n = hf * NHH + q
                            nc.tensor.matmul(
                                pt[:, q * CHK : (q + 1) * CHK],
                                s_wm[:, col : col + 128],
                                gcur[:, kc, n * CHK : (n + 1) * CHK],
                                start=(kc == 0),
                                stop=(kc == KC - 1),
                            )
                    nc.scalar.activation(
                        st[f"g{l + 1}"][:, fc, hf * HP : (hf + 1) * HP],
                        pt[:],
                        AF.Tanh,
                        bias=s_bm[:, bcol : bcol + 1],
                    )

            def sub_last_drain(ts, c, st):
                # last layer into [1, PTILE] psum, then DVE folds the softmax
                # weight in and the row is DMAd into the tile's u assembly
                gcur = st[f"g{SNMID}"]
                ur = rp.tile([1, PTILE], f32r, tag="ur", name="ur")
                for hf in range(2):
                    pu = pp.tile([1, HP], f32, tag="mm", name="pu")
                    for kc in range(KC):
                        wcol = c * KC + kc
                        for q in range(NHH):
                            n = hf * NHH + q
                            nc.tensor.matmul(
                                pu[:, q * CHK : (q + 1) * CHK],
                                s_wl[:, wcol : wcol + 1],
                                gcur[:, kc, n * CHK : (n + 1) * CHK],
                                start=(kc == 0),
                                stop=(kc == KC - 1),
                            )
                    nc.vector.tensor_copy(ur[:, hf * HP : (hf + 1) * HP], pu[:])
                nc.sync.dma_start(ts["ua"][c : c + 1, :], ur[:])

            def combine(ts):
                t = ts["t"]
                # fold softmax weights into the assembled u rows (16
                # partitions, base 0 — aligned for the DVE)
                nc.vector.tensor_mul(ts["ua"][:], ts["ua"][:], ts["ez"][:])
                accrow = rp.tile([2, PTILE], f32, tag="row", name="accrow")
                for hf in range(2):
                    pacc = pp.tile([1, HP], f32, tag="mm", name="pacc")
                    for q in range(NHH):
                        n = hf * NHH + q
                        nc.tensor.matmul(
                            pacc[:, q * CHK : (q + 1) * CHK],
                            s_one[:],
                            ts["ua"][:, n * CHK : (n + 1) * CHK],
                            start=True,
                            stop=True,
                        )
                    nc.vector.tensor_copy(accrow[0:1, hf * HP : (hf + 1) * HP], pacc[:])
                nc.sync.dma_start(s_accP[t * 32 : (t + 1) * 32, :], accrow[0:1, :])

            def chain_step(j, k):
                t, c = divmod(j, C)
                ts = tiles[t]
                st = ts["st"].setdefault(c, {})
                if k <= 1:
                    sub_l0(ts, c, st, k)
                elif k <= 5:
                    sub_mid(c, st, (k - 2) // FC, (k - 2) % FC)
                else:
                    sub_last_drain(ts, c, st)

            def make_tile(t):
                def h():
                    ts = {"t": t, "st": {}}
                    ts["xt4"] = xt4_0 if t == 0 else xfetched.pop(t)
                    ts["ua"] = sp.tile([C, PTILE], f32r, tag="ua", name="ua")
                    tiles[t] = ts
                return h

            def make_prefetch(t):
                def h():
                    nxt = xpl.tile([128, PTILE], f32r, tag="xt", name="xt4")
                    for rr in range(NCH):
                        nc.sync.dma_start(
                            nxt[32 * rr : 32 * rr + 3, rr * CHK : (rr + 1) * CHK],
                            xT[:, t * PTILE + rr * CHK : t * PTILE + (rr + 1) * CHK],
                        )
                    xfetched[t] = nxt
                return h

            hooks = {}

            def add_hook(s, fn):
                hooks.setdefault(s, []).append(fn)

            for t in range(NT):
                base = OFF * C * t
                add_hook(base + 0, make_tile(t))
                add_hook(base + 0, lambda t=t: pou_l0(tiles[t]))
                add_hook(base + 3, lambda t=t: pou_mid(tiles[t], 0))
                add_hook(base + 6, lambda t=t: pou_mid(tiles[t], 1))
                add_hook(base + 9, lambda t=t: pou_logits(tiles[t]))
                add_hook(base + 12, lambda t=t: pou_s12(tiles[t]))
                if t + 1 < NT:
                    add_hook(base + 30, make_prefetch(t + 1))
                # combine right after this tile's final drain
                add_hook(OFF * (t * C + C - 1) + 7, lambda t=t: combine(tiles[t]))

            total_slots = OFF * (NJ - 1) + 8
            for s in range(total_slots):
                for fn in hooks.get(s, ()):
                    fn()
                jmin = max(0, (s - 6 + OFF - 1) // OFF)
                jmax = min(NJ - 1, s // OFF)
                for j in range(jmin, jmax + 1):
                    k = s - OFF * j
                    if 0 <= k <= 6:
                        chain_step(j, k)

            # ---------- final: combine in points-layout ----------
            s_r = fin.tile([128, PPB], f32)
            nc.vector.reciprocal(s_r[:], s_s1P[:])
            s_num = fin.tile([128, PPB], f32)
            nc.vector.tensor_add(s_num[:], s_accP[:], s_s2P[:])
            nc.vector.tensor_mul(s_num[:], s_num[:], s_r[:])
            nc.vector.tensor_mul(s_num[:], s_num[:], s_bc[:])
            nc.sync.dma_start(y.rearrange("(p j) -> p j", p=128), s_num[:])

    nc.compile()
    return nc


def _prep_inputs(inputs):
    f = lambda k: np.ascontiguousarray(np.asarray(inputs[k]), dtype=np.float32)
    x = f("x")
    centers, scales = f("centers"), f("scales")
    sub_W0, sub_b0 = f("sub_W0"), f("sub_b0")
    sub_Wmid, sub_bmid = f("sub_Wmid"), f("sub_bmid")
    sub_Wl, sub_bl = f("sub_Wl"), f("sub_bl")

    # fold per-subdomain normalization into layer-0 weights:
    # xn = (x - c)/s  =>  xn @ W0 + b0 = x @ (W0/s) + (b0 - (c/s) @ W0)
    w0e_full = sub_W0 / scales[:, :, None]                       # [C, D, SW]
    b0e_full = sub_b0 - np.einsum("cd,cdw->cw", centers / scales, sub_W0)

    # row-group packed layer-0 weights: rows {32r,32r+1,32r+2} = [W0; W1; b]
    w0q = np.zeros((128, C * FC * 128), np.float32)
    for c in range(C):
        for fc in range(FC):
            col = (c * FC + fc) * 128
            blk = np.vstack(
                [
                    w0e_full[c][:, fc * 128 : (fc + 1) * 128],
                    b0e_full[c][None, fc * 128 : (fc + 1) * 128],
                ]
            )
            for rr in range(4):
                w0q[32 * rr : 32 * rr + 3, col : col + 128] = blk

    wm = np.ascontiguousarray(
        sub_Wmid.reshape(C, SNMID, KC, 128, FC, 128)
        .transpose(3, 0, 1, 4, 2, 5)
        .reshape(128, C * WBLK)
    ).astype(np.float16)
    bm = np.ascontiguousarray(
        sub_bmid.reshape(C, SNMID, FC, 128).transpose(3, 0, 1, 2).reshape(128, -1)
    )
    wl = np.ascontiguousarray(
        sub_Wl.reshape(C, KC, 128).transpose(2, 0, 1).reshape(128, -1)
    ).astype(np.float16)
    cwm = np.ascontiguousarray(
        np.stack([np.ones(C, np.float32), sub_bl[:, 0]], axis=1)
    ).astype(np.float16)

    pw0q = np.zeros((128, PH), np.float32)
    pblk = np.vstack([f("pou_W0"), f("pou_b0")[None, :]])
    for rr in range(4):
        pw0q[32 * rr : 32 * rr + 3, :] = pblk

    shared = dict(
        w0q=w0q,
        wm=wm,
        bm=bm,
        wl=wl,
        pw0q=pw0q,
        pwm=np.ascontiguousarray(f("pou_Wmid").transpose(1, 0, 2).reshape(PH, -1)),
        pbm=np.ascontiguousarray(f("pou_bmid").T),
        pwl=f("pou_Wl"),
        pbl=np.ascontiguousarray(f("pou_bl")[:, None]),
        cw=cwm,
        onec=np.ones((C, 1), np.float32),
    )

    in_maps = []
    for core in range(NCORES):
        xs = x[core * NP : (core + 1) * NP]
        m = dict(shared)
        m["xT"] = np.ascontiguousarray(
            np.vstack([xs.T, np.ones((1, NP), np.float32)])
        )
        m["xP"] = np.ascontiguousarray(xs.reshape(128, 2 * PPB))
        in_maps.append(m)
    return in_maps


def kernel(**inputs):
    from concourse.bass_utils import run_bass_kernel_spmd

    if "nc" not in _CACHE:
        _CACHE["nc"] = _build()
    nc = _CACHE["nc"]

    in_maps = _prep_inputs(inputs)
    trace = os.environ.get("KERNEL_TRACE", "0") == "1"
    res = run_bass_kernel_spmd(
        nc, in_maps, core_ids=list(range(NCORES)), trace=trace
    )
    kernel.last_results = res
    y = np.concatenate([res.results[i]["y"] for i in range(NCORES)])
    return y.astype(np.float32)


# revision 31
# speedup vs baseline: 1.0197x; 1.0197x over previous
"""Trainium2 Bass kernel for FBPINN-with-window (dense MoE over 16 subnets).

Math (per point n):
    h   = relu(x @ pW0 + pb0); h += relu(h @ pWmid_l + pbmid_l) (x2)
    z   = h @ pWl + pbl;  ez = exp(z)            (softmax un-normalized)
    xn_c = (x - center_c)/scale_c  (folded on host into layer-0 weights)
    g_c = tanh(xn_c @ W0_c + b0_c); g_c = tanh(g_c @ Wmid_cl + bmid_cl) (x2)
    u_c = g_c @ Wl_c + bl_c
    acc = sum_c softmax(z)_c * u_c = (sum_c ez_c*(g_c@Wl_c) + sum_c ez_c*bl_c) / sum_c ez_c
    out = acc * x0(1-x0)*x1(1-x1)

Device layout: activations transposed (features on partitions, points on the
free dim).  Data-parallel over 8 cores (8192 points each), 4 point-tiles of
2048 per core.  Subnet activations and mid/last weights are fp16 (rel err
~5e-4 vs fp32 — verified against the reference on host); all weights are
SBUF-resident, no streaming.

The Scalar (tanh) and Tensor engines are both ~790us of intrinsic work per
core, so the schedule keeps them co-saturated: PSUM is run as a ring of four
[128, 1024] half-tiles (one PE half-fill ~1.06us matches one half-tanh
~1.02us), and the per-subnet 7-step chains (l0 fc0/fc1, 4 mids, last) are
software-pipelined with a 3-slot skew so every window mixes ACT-heavy l0,
balanced mid, and PE-only last steps from different subnets.  Last layers
write [1, PTILE] PSUM rows that a DVE multiply folds the softmax weight into.
"""

import os

import numpy as np

N = 65536
D = 2
C = 16
PH = 128
PNMID = 2
SW = 256
SNMID = 2

NCORES = 8
NP = N // NCORES          # 8192 points per core
PTILE = 2048              # points per tile
NT = NP // PTILE          # 4 tiles
CHK = 512                 # matmul moving free dim (one PSUM bank)
NCH = PTILE // CHK        # 4 chunks per tile
FC = SW // 128            # 2 feature chunks
KC = SW // 128            # 2 contraction chunks
PPB = NP // 128           # 64 points per partition (points-layout)
HP = PTILE // 2           # psum half-tile (2 banks): 4-slot psum ring
NHH = HP // CHK           # 2 chunks per half
WBLK = SNMID * FC * KC * 128  # mid-weight cols per subnet (1024)

_CACHE = {}


def _build():
    import concourse.mybir as mybir
    import concourse.tile as tile
    from concourse import bacc

    f32 = mybir.dt.float32
    f32r = mybir.dt.float32r
    f16 = mybir.dt.float16
    AF = mybir.ActivationFunctionType
    OP = mybir.AluOpType

    nc = bacc.Bacc("TRN2", debug=False)

    def din(name, shape, dt=f32):
        return nc.dram_tensor(name, shape, dt, kind="ExternalInput").ap()

    xT = din("xT", (3, NP), f32r)
    xP = din("xP", (128, 2 * PPB))
    w0q = din("w0q", (128, C * FC * 128), f32r)
    wm = din("wm", (128, C * WBLK), f16)
    bm = din("bm", (128, C * SNMID * FC))
    wl = din("wl", (128, C * KC), f16)
    pw0q = din("pw0q", (128, PH), f32r)
    pwm = din("pwm", (PH, PNMID * PH), f32r)
    pbm = din("pbm", (PH, PNMID))
    pwl = din("pwl", (PH, C), f32r)
    pbl = din("pbl", (C, 1))
    cw = din("cw", (C, 2), f16)
    onec = din("onec", (C, 1), f32r)
    y = nc.dram_tensor("y", (NP,), f32, kind="ExternalOutput").ap()

    with tile.TileContext(nc) as tc:
        with (
            tc.tile_pool(name="wp", bufs=1) as wp,
            tc.tile_pool(name="gp", bufs=6) as gp,
            tc.tile_pool(name="hp", bufs=2) as hp,
            tc.tile_pool(name="sp", bufs=2) as sp,
            tc.tile_pool(name="rp", bufs=2) as rp,
            tc.tile_pool(name="xp", bufs=2) as xpl,
            tc.tile_pool(name="fin", bufs=1) as fin,
            tc.tile_pool(name="pp", bufs=4, space="PSUM") as pp,
        ):
            # ---- weights / constants into SBUF (all resident) ----
            s_pw0q = wp.tile([128, PH], f32r)
            nc.sync.dma_start(s_pw0q[:], pw0q)
            s_pwm = wp.tile([PH, PNMID * PH], f32r)
            nc.sync.dma_start(s_pwm[:], pwm)
            s_pbm = wp.tile([PH, PNMID], f32)
            nc.sync.dma_start(s_pbm[:], pbm)
            s_pwl = wp.tile([PH, C], f32r)
            nc.sync.dma_start(s_pwl[:], pwl)
            s_pbl = wp.tile([C, 1], f32)
            nc.sync.dma_start(s_pbl[:], pbl)
            s_cw = wp.tile([C, 2], f16)
            nc.sync.dma_start(s_cw[:], cw)
            s_one = wp.tile([C, 1], f32r)
            nc.sync.dma_start(s_one[:], onec)
            s_w0q = wp.tile([128, C * FC * 128], f32r)
            for cq in range(4):
                qb = C * FC * 32
                nc.sync.dma_start(
                    s_w0q[:, cq * qb : (cq + 1) * qb], w0q[:, cq * qb : (cq + 1) * qb]
                )
            # tile-0 x layout before the big mid-weight load: it is on the
            # critical path of the first l0 matmuls
            xt4_0 = wp.tile([128, PTILE], f32r)
            for rr in range(NCH):
                nc.sync.dma_start(
                    xt4_0[32 * rr : 32 * rr + 3, rr * CHK : (rr + 1) * CHK],
                    xT[:, rr * CHK : (rr + 1) * CHK],
                )
            s_bm = wp.tile([128, C * SNMID * FC], f32)
            nc.sync.dma_start(s_bm[:], bm)
            s_wl = wp.tile([128, C * KC], f16)
            nc.sync.dma_start(s_wl[:], wl)
            s_wm = wp.tile([128, C * WBLK], f16)
            for cq in range(16):
                qb = C * WBLK // 16
                nc.sync.dma_start(
                    s_wm[:, cq * qb : (cq + 1) * qb], wm[:, cq * qb : (cq + 1) * qb]
                )

            # ---- per-core x (points-layout) + boundary factor ----
            s_xP = fin.tile([128, 2 * PPB], f32)
            nc.sync.dma_start(s_xP[:], xP)
            s_xmx = fin.tile([128, 2 * PPB], f32)
            nc.vector.tensor_mul(s_xmx[:], s_xP[:], s_xP[:])
            nc.vector.tensor_sub(s_xmx[:], s_xP[:], s_xmx[:])
            v = s_xmx.rearrange("p (j two) -> p j two", two=2)
            s_bc = fin.tile([128, PPB], f32)
            nc.vector.tensor_mul(s_bc[:], v[:, :, 0], v[:, :, 1])

            # points-layout accumulators, filled per tile via reshape DMAs
            s_accP = fin.tile([128, PPB], f32)
            s_s1P = fin.tile([128, PPB], f32)
            s_s2P = fin.tile([128, PPB], f32)

            # ---------- skewed software pipeline over (tile, subnet) chains ----
            # Chain j = t*C + c runs its 7 steps (l0 fc0/fc1, 4 mids, last)
            # at slots OFF*j + k.  Each slot then mixes l0 (ACT-heavy), mid
            # (balanced) and last (PE-only) steps of different subnets, so
            # both engines see near-constant load instead of phase bursts.
            OFF = 3
            NJ = NT * C
            tiles = {}
            xfetched = {}

            def pou_l0(ts):
                h = hp.tile([PH, PTILE], f32r, tag="h", name="h")
                for hf in range(2):
                    ps0 = pp.tile([PH, HP], f32, tag="mm", name="ps0")
                    for q in range(NHH):
                        rr = hf * NHH + q
                        nc.tensor.matmul(
                            ps0[:, q * CHK : (q + 1) * CHK],
                            s_pw0q[32 * rr : 32 * rr + 3, :],
                            ts["xt4"][32 * rr : 32 * rr + 3, rr * CHK : (rr + 1) * CHK],
                            start=True,
                            stop=True,
                            tile_position=(32 * rr, 0),
                        )
                    nc.vector.tensor_scalar_max(h[:, hf * HP : (hf + 1) * HP], ps0[:], 0.0)
                ts["h"] = h

            def pou_mid(ts, l):
                h = ts["h"]
                hr = hp.tile([PH, PTILE], f32r, tag="h", name="hr")
                for hf in range(2):
                    psl = pp.tile([PH, HP], f32, tag="mm", name="psl")
                    for q in range(NHH):
                        n = hf * NHH + q
                        nc.tensor.matmul(
                            psl[:, q * CHK : (q + 1) * CHK],
                            s_pwm[:, l * PH : (l + 1) * PH],
                            h[:, n * CHK : (n + 1) * CHK],
                            start=True,
                            stop=True,
                        )
                    nc.vector.tensor_scalar(
                        hr[:, hf * HP : (hf + 1) * HP], psl[:],
                        s_pbm[:, l : l + 1], 0.0, op0=OP.add, op1=OP.max
                    )
                nc.vector.tensor_add(hr[:], hr[:], h[:])
                ts["h"] = hr

            def pou_logits(ts):
                h = ts["h"]
                ez = sp.tile([C, PTILE], f16, tag="ez", name="ez")
                for hf in range(2):
                    psz = pp.tile([C, HP], f32, tag="mm", name="psz")
                    for q in range(NHH):
                        n = hf * NHH + q
                        nc.tensor.matmul(
                            psz[:, q * CHK : (q + 1) * CHK],
                            s_pwl[:],
                            h[:, n * CHK : (n + 1) * CHK],
                            start=True,
                            stop=True,
                        )
                    nc.scalar.activation(
                        ez[:, hf * HP : (hf + 1) * HP], psz[:], AF.Exp,
                        bias=s_pbl[:, 0:1],
                    )
                ts["ez"] = ez

            def pou_s12(ts):
                t = ts["t"]
                s12row = rp.tile([2, PTILE], f32, tag="row", name="s12row")
                for hf in range(2):
                    pss = pp.tile([2, HP], f32, tag="mm", name="pss")
                    for q in range(NHH):
                        n = hf * NHH + q
                        nc.tensor.matmul(
                            pss[:, q * CHK : (q + 1) * CHK],
                            s_cw[:],
                            ts["ez"][:, n * CHK : (n + 1) * CHK],
                            start=True,
                            stop=True,
                        )
                    nc.vector.tensor_copy(s12row[:, hf * HP : (hf + 1) * HP], pss[:])
                nc.sync.dma_start(s_s1P[t * 32 : (t + 1) * 32, :], s12row[0:1, :])
                nc.sync.dma_start(s_s2P[t * 32 : (t + 1) * 32, :], s12row[1:2, :])

            def sub_l0(ts, c, st, fc):
                if fc == 0:
                    st["g0"] = gp.tile([128, KC, PTILE], f16, tag="g", name="g0")
                col = (c * FC + fc) * 128
                for hf in range(2):
                    pt = pp.tile([128, HP], f32, tag="mm", name="pt")
                    for q in range(NHH):
                        rr = hf * NHH + q
                        nc.tensor.matmul(
                            pt[:, q * CHK : (q + 1) * CHK],
                            s_w0q[32 * rr : 32 * rr + 3, col : col + 128],
                            ts["xt4"][32 * rr : 32 * rr + 3, rr * CHK : (rr + 1) * CHK],
                            start=True,
                            stop=True,
                            tile_position=(32 * rr, 0),
                        )
                    nc.scalar.activation(
                        st["g0"][:, fc, hf * HP : (hf + 1) * HP], pt[:], AF.Tanh
                    )

            def sub_mid(c, st, l, fc):
                gcur = st[f"g{l}"]
                if fc == 0:
                    st[f"g{l + 1}"] = gp.tile(
                        [128, KC, PTILE], f16, tag="g", name=f"g{l + 1}"
                    )
                bcol = (c * SNMID + l) * FC + fc
                for hf in range(2):
                    pt = pp.tile([128, HP], f32, tag="mm", name="pt")
                    for kc in range(KC):
                        col = c * WBLK + ((l * FC + fc) * KC + kc) * 128
                        for q in range(NHH):
                            n = hf * NHH + q
                            nc.tensor.matmul(
                                pt[:, q * CHK : (q + 1) * CHK],
                                s_wm[:, col : col + 128],
                                gcur[:, kc, n * CHK : (n + 1) * CHK],
                                start=(kc == 0),
                                stop=(kc == KC - 1),
                            )
                    nc.scalar.activation(
                        st[f"g{l + 1}"][:, fc, hf * HP : (hf + 1) * HP],
                        pt[:],
                        AF.Tanh,
                        bias=s_bm[:, bcol : bcol + 1],
                    )

            def sub_last_drain(ts, c, st):
                # last layer into [1, PTILE] psum, then DVE folds the softmax
                # weight in and the row is DMAd into the tile's u assembly
                gcur = st[f"g{SNMID}"]
                ur = rp.tile([1, PTILE], f32r, tag="ur", name="ur")
                for hf in range(2):
                    pu = pp.tile([1, HP], f32, tag="mm", name="pu")
                    for kc in range(KC):
                        wcol = c * KC + kc
                        for q in range(NHH):
                            n = hf * NHH + q
                            nc.tensor.matmul(
                                pu[:, q * CHK : (q + 1) * CHK],
                                s_wl[:, wcol : wcol + 1],
                                gcur[:, kc, n * CHK : (n + 1) * CHK],
                                start=(kc == 0),
                                stop=(kc == KC - 1),
                            )
                    nc.vector.tensor_mul(
                        ur[:, hf * HP : (hf + 1) * HP], pu[:],
                        st["ezp"][:, hf * HP : (hf + 1) * HP],
                    )
                nc.sync.dma_start(ts["ua"][c : c + 1, :], ur[:])

            def combine(ts):
                t = ts["t"]
                accrow = rp.tile([2, PTILE], f32, tag="row", name="accrow")
                for hf in range(2):
                    pacc = pp.tile([1, HP], f32, tag="mm", name="pacc")
                    for q in range(NHH):
                        n = hf * NHH + q
                        nc.tensor.matmul(
                            pacc[:, q * CHK : (q + 1) * CHK],
                            s_one[:],
                            ts["ua"][:, n * CHK : (n + 1) * CHK],
                            start=True,
                            stop=True,
                        )
                    nc.vector.tensor_copy(accrow[0:1, hf * HP : (hf + 1) * HP], pacc[:])
                nc.sync.dma_start(s_accP[t * 32 : (t + 1) * 32, :], accrow[0:1, :])

            def chain_step(j, k):
                t, c = divmod(j, C)
                ts = tiles[t]
                st = ts["st"].setdefault(c, {})
                if k <= 1:
                    sub_l0(ts, c, st, k)
                elif k <= 5:
                    sub_mid(c, st, (k - 2) // FC, (k - 2) % FC)
                    if k == 5:
                        # stage this chain's ez row at partition 0 for the
                        # drain (DVE reads need 32-aligned partition bases)
                        ezp = rp.tile([1, PTILE], f16, tag="ezp", name="ezp")
                        nc.sync.dma_start(ezp[:], ts["ez"][c : c + 1, :])
                        st["ezp"] = ezp
                else:
                    sub_last_drain(ts, c, st)

            def make_tile(t):
                def h():
                    ts = {"t": t, "st": {}}
                    ts["xt4"] = xt4_0 if t == 0 else xfetched.pop(t)
                    ts["ua"] = sp.tile([C, PTILE], f32r, tag="ua", name="ua")
                    tiles[t] = ts
                return h

            def make_prefetch(t):
                def h():
                    nxt = xpl.tile([128, PTILE], f32r, tag="xt", name="xt4")
                    for rr in range(NCH):
                        nc.sync.dma_start(
                            nxt[32 * rr : 32 * rr + 3, rr * CHK : (rr + 1) * CHK],
                            xT[:, t * PTILE + rr * CHK : t * PTILE + (rr + 1) * CHK],
                        )
                    xfetched[t] = nxt
                return h

            hooks = {}

            def add_hook(s, fn):
                hooks.setdefault(s, []).append(fn)

            for t in range(NT):
                base = OFF * C * t
                add_hook(base + 0, make_tile(t))
                add_hook(base + 0, lambda t=t: pou_l0(tiles[t]))
                add_hook(base + 1, lambda t=t: pou_mid(tiles[t], 0))
                add_hook(base + 2, lambda t=t: pou_mid(tiles[t], 1))
                add_hook(base + 3, lambda t=t: pou_logits(tiles[t]))
                add_hook(base + 4, lambda t=t: pou_s12(tiles[t]))
                if t + 1 < NT:
                    add_hook(base + 30, make_prefetch(t + 1))
                # combine right after this tile's final drain
                add_hook(OFF * (t * C + C - 1) + 7, lambda t=t: combine(tiles[t]))

            total_slots = OFF * (NJ - 1) + 8
            for s in range(total_slots):
                for fn in hooks.get(s, ()):
                    fn()
                jmin = max(0, (s - 6 + OFF - 1) // OFF)
                jmax = min(NJ - 1, s // OFF)
                for j in range(jmin, jmax + 1):
                    k = s - OFF * j
                    if 0 <= k <= 6:
                        chain_step(j, k)

            # ---------- final: combine in points-layout ----------
            s_r = fin.tile([128, PPB], f32)
            nc.vector.reciprocal(s_r[:], s_s1P[:])
            s_num = fin.tile([128, PPB], f32)
            nc.vector.tensor_add(s_num[:], s_accP[:], s_s2P[:])
            nc.vector.tensor_mul(s_num[:], s_num[:], s_r[:])
            nc.vector.tensor_mul(s_num[:], s_num[:], s_bc[:])
            nc.sync.dma_start(y.rearrange("(p j) -> p j", p=128), s_num[:])

    nc.compile()
    return nc


def _prep_inputs(inputs):
    f = lambda k: np.ascontiguousarray(np.asarray(inputs[k]), dtype=np.float32)
    x = f("x")
    centers, scales = f("centers"), f("scales")
    sub_W0, sub_b0 = f("sub_W0"), f("sub_b0")
    sub_Wmid, sub_bmid = f("sub_Wmid"), f("sub_bmid")
    sub_Wl, sub_bl = f("sub_Wl"), f("sub_bl")

    # fold per-subdomain normalization into layer-0 weights:
    # xn = (x - c)/s  =>  xn @ W0 + b0 = x @ (W0/s) + (b0 - (c/s) @ W0)
    w0e_full = sub_W0 / scales[:, :, None]                       # [C, D, SW]
    b0e_full = sub_b0 - np.einsum("cd,cdw->cw", centers / scales, sub_W0)

    # row-group packed layer-0 weights: rows {32r,32r+1,32r+2} = [W0; W1; b]
    w0q = np.zeros((128, C * FC * 128), np.float32)
    for c in range(C):
        for fc in range(FC):
            col = (c * FC + fc) * 128
            blk = np.vstack(
                [
                    w0e_full[c][:, fc * 128 : (fc + 1) * 128],
                    b0e_full[c][None, fc * 128 : (fc + 1) * 128],
                ]
            )
            for rr in range(4):
                w0q[32 * rr : 32 * rr + 3, col : col + 128] = blk

    wm = np.ascontiguousarray(
        sub_Wmid.reshape(C, SNMID, KC, 128, FC, 128)
        .transpose(3, 0, 1, 4, 2, 5)
        .reshape(128, C * WBLK)
    ).astype(np.float16)
    bm = np.ascontiguousarray(
        sub_bmid.reshape(C, SNMID, FC, 128).transpose(3, 0, 1, 2).reshape(128, -1)
    )
    wl = np.ascontiguousarray(
        sub_Wl.reshape(C, KC, 128).transpose(2, 0, 1).reshape(128, -1)
    ).astype(np.float16)
    cwm = np.ascontiguousarray(
        np.stack([np.ones(C, np.float32), sub_bl[:, 0]], axis=1)
    ).astype(np.float16)

    pw0q = np.zeros((128, PH), np.float32)
    pblk = np.vstack([f("pou_W0"), f("pou_b0")[None, :]])
    for rr in range(4):
        pw0q[32 * rr : 32 * rr + 3, :] = pblk

    shared = dict(
        w0q=w0q,
        wm=wm,
        bm=bm,
        wl=wl,
        pw0q=pw0q,
        pwm=np.ascontiguousarray(f("pou_Wmid").transpose(1, 0, 2).reshape(PH, -1)),
        pbm=np.ascontiguousarray(f("pou_bmid").T),
        pwl=f("pou_Wl"),
        pbl=np.ascontiguousarray(f("pou_bl")[:, None]),
        cw=cwm,
        onec=np.ones((C, 1), np.float32),
    )

    in_maps = []
    for core in range(NCORES):
        xs = x[core * NP : (core + 1) * NP]
        m = dict(shared)
        m["xT"] = np.ascontiguousarray(
            np.vstack([xs.T, np.ones((1, NP), np.float32)])
        )
        m["xP"] = np.ascontiguousarray(xs.reshape(128, 2 * PPB))
        in_maps.append(m)
    return in_maps


def kernel(**inputs):
    from concourse.bass_utils import run_bass_kernel_spmd

    if "nc" not in _CACHE:
        _CACHE["nc"] = _build()
    nc = _CACHE["nc"]

    in_maps = _prep_inputs(inputs)
    trace = os.environ.get("KERNEL_TRACE", "0") == "1"
    res = run_bass_kernel_spmd(
        nc, in_maps, core_ids=list(range(NCORES)), trace=trace
    )
    kernel.last_results = res
    y = np.concatenate([res.results[i]["y"] for i in range(NCORES)])
    return y.astype(np.float32)
